# revision 9
# baseline (speedup 1.0000x reference)
"""Trainium2 Bass kernel for nn_ConvEmbedder.

out[b, p, e] = (patch(b, p) . conv_w + conv_b) * lin_w[e] + lin_b[e]

data [64, 512, 512] f32, non-overlapping 16x16 patches (1024 per image),
E = 768.  Pure data-parallel over the batch: 8 images per NeuronCore.

Per-core pipeline (per image):
  1. DMA image -> SBUF [128, 4, 512]   (row-group g holds image rows 128g..128g+127)
  2. DVE: tmp = d * wexp               (wexp[p, g, w] = conv_w[p%16, w%16])
  3. PE: 4 accumulating matmuls with block-diagonal ones lhsT -> PSUM t2[32, 512]
       t2[i, 16j+c] = sum_r tmp[row=16*i+r, 16j+c]   (i = absolute patch row)
  4. DVE reduce over c-groups: v[32, 32],  v[i, j] = conv_val(patch(i, j)) - conv_b
  5. SBUF->SBUF DMA flattens v to lhsT[0, 0:1024] (patch-major); lhsT[1, :] = 1.0
  6. PE per 128-patch block: [1; v].T @ [lin_b + conv_b*lin_w; lin_w] -> PSUM [128, 768]
  7. PSUM->SBUF copy split between DVE and ScalarE
  8. contiguous 768 KB DMA store per 2 blocks
"""

import numpy as np

import concourse.bacc as bacc
import concourse.tile as tile
from concourse import mybir
from concourse.bass_utils import run_bass_kernel_spmd

KS = 16          # conv kernel == patch size
E = 768          # embed dim
NCORES = 8
B = 64
H = 512
W = 512
BPC = B // NCORES          # images per core
NH = H // KS               # 32 patch rows (and patch cols) per image
NPATCH = NH * NH           # 1024 patches per image
NG = H // 128              # 4 row-groups per image
NBLK = NPATCH // 128       # 8 expansion blocks of 128 patches
DT = mybir.dt.float32
SPLIT = 352                # DVE copies cols [0:SPLIT), ScalarE [SPLIT:768)

_NC = None
_LAST_RESULTS = None       # BassKernelResults of the last run (for test harness)


def _build_nc(reps=None):
    # reps: bench-only — wrap the whole per-image pipeline in a HW For_i loop
    # so one NEFF launch amortizes the (huge) axon dispatch overhead.
    nc = bacc.Bacc("TRN2", target_bir_lowering=False, debug=False)
    data_t = nc.dram_tensor("data", [BPC, H, W], DT, kind="ExternalInput")
    wexp_t = nc.dram_tensor("wexp", [128, NG, W], DT, kind="ExternalInput")
    bd_t = nc.dram_tensor("bd", [128, NG, NH], DT, kind="ExternalInput")
    lwb_t = nc.dram_tensor("lwb", [2, E], DT, kind="ExternalInput")
    out_t = nc.dram_tensor("out", [BPC, NPATCH, E], DT, kind="ExternalOutput")

    with tile.TileContext(nc) as tc:
        with (
            tc.tile_pool(name="singles", bufs=1) as singles,
            tc.tile_pool(name="dpool", bufs=2) as dpool,
            tc.tile_pool(name="tpool", bufs=2) as tpool,
            tc.tile_pool(name="vpool", bufs=2) as vpool,
            tc.tile_pool(name="lpool", bufs=2) as lpool,
            tc.tile_pool(name="opool", bufs=4) as opool,
            tc.tile_pool(name="pt2p", bufs=2, space="PSUM") as pt2p,
            tc.tile_pool(name="pop", bufs=3, space="PSUM") as pop,
        ):
            wexp = singles.tile([128, NG, W], DT)
            nc.sync.dma_start(out=wexp[:], in_=wexp_t.ap())
            bd = singles.tile([128, NG, NH], DT)
            nc.sync.dma_start(out=bd[:], in_=bd_t.ap())
            lwb = singles.tile([2, E], DT)
            nc.sync.dma_start(out=lwb[:], in_=lwb_t.ap())

            def emit_images():
              for img in range(BPC):
                # 1. load image: d[p, g, w] = data[img, 128*g + p, w]
                d = dpool.tile([128, NG, W], DT)
                nc.sync.dma_start(
                    out=d[:],
                    in_=data_t.ap()[img].rearrange("(g p) w -> p g w", p=128),
                )
                # 2. elementwise conv-weight multiply
                tmp = tpool.tile([128, NG, W], DT)
                nc.vector.tensor_mul(tmp[:], d[:], wexp[:])
                # 3. reduce the 16 rows of each patch-row via block-diag ones
                pt2 = pt2p.tile([NH, W], DT)
                for g in range(NG):
                    nc.tensor.matmul(
                        pt2[:],
                        bd[:, g, :],
                        tmp[:, g, :],
                        start=(g == 0),
                        stop=(g == NG - 1),
                    )
                # 4. reduce the 16 cols of each patch
                v8 = vpool.tile([NH, NH], DT)
                nc.vector.tensor_reduce(
                    out=v8[:],
                    in_=pt2[:].rearrange("i (j c) -> i j c", c=KS),
                    axis=mybir.AxisListType.X,
                    op=mybir.AluOpType.add,
                )
                # 5. row 0 = ones, row 1 = v flattened patch-major
                lhsT = lpool.tile([2, NPATCH], DT)
                nc.gpsimd.memset(lhsT[0:1, :], 1.0)
                nc.sync.dma_start(
                    out=lhsT[1:2, :].rearrange("o (i j) -> o i j", j=NH),
                    in_=v8[:],
                )
                # 6-8. expansion: out[p, e] = v[p]*lin_w[e] + lin_b_eff[e]
                for bb in range(0, NBLK, 2):
                    ot = opool.tile([128, 2, E], DT)
                    for sub in range(2):
                        blk = bb + sub
                        lhsT_blk = lhsT[:, 128 * blk:128 * (blk + 1)]
                        po = pop.tile([128, E], DT)
                        nc.tensor.matmul(
                            po[:, 0:512], lhsT_blk, lwb[:, 0:512],
                            start=True, stop=True,
                        )
                        nc.tensor.matmul(
                            po[:, 512:E], lhsT_blk, lwb[:, 512:E],
                            start=True, stop=True,
                        )
                        nc.vector.tensor_copy(ot[:, sub, 0:SPLIT], po[:, 0:SPLIT])
                        nc.scalar.copy(ot[:, sub, SPLIT:E], po[:, SPLIT:E])
                    nc.scalar.dma_start(
                        out=out_t.ap()[img, 128 * bb:128 * (bb + 2), :]
                        .rearrange("(blk p) e -> p blk e", p=128),
                        in_=ot[:],
                    )

            if reps is None:
                emit_images()
            else:
                with tc.For_i(0, reps, 1):
                    emit_images()
    nc.compile()
    return nc


def _get_nc():
    global _NC
    if _NC is None:
        _NC = _build_nc()
    return _NC


def _prepare_in_maps(data, conv_w, conv_b, lin_w, lin_b):
    data = np.ascontiguousarray(np.asarray(data, dtype=np.float32))
    conv_w = np.asarray(conv_w, dtype=np.float32).reshape(KS, KS)
    conv_b = np.float32(np.asarray(conv_b, dtype=np.float32))
    lin_w = np.asarray(lin_w, dtype=np.float32).reshape(E)
    lin_b = np.asarray(lin_b, dtype=np.float32).reshape(E)

    # wexp[p, g, w] = conv_w[p % 16, w % 16]
    wexp = np.ascontiguousarray(
        np.broadcast_to(
            np.tile(conv_w, (128 // KS, W // KS))[:, None, :], (128, NG, W)
        )
    )
    # bd[row, g, m] = 1 iff m == 8*g + row//16  (block-diagonal ones)
    bd = np.zeros((128, NG, NH), dtype=np.float32)
    rows = np.arange(128)
    for g in range(NG):
        bd[rows, g, (128 // KS) * g + rows // KS] = 1.0
    # fold conv_b: v*lin_w + (conv_b*lin_w + lin_b)
    lin_b_eff = (
        np.float64(conv_b) * lin_w.astype(np.float64) + lin_b.astype(np.float64)
    ).astype(np.float32)
    lwb = np.ascontiguousarray(np.stack([lin_b_eff, lin_w], axis=0))

    return [
        {
            "data": np.ascontiguousarray(data[i * BPC:(i + 1) * BPC]),
            "wexp": wexp,
            "bd": bd,
            "lwb": lwb,
        }
        for i in range(NCORES)
    ]


def kernel(data, conv_w, conv_b, lin_w, lin_b):
    global _LAST_RESULTS
    in_maps = _prepare_in_maps(data, conv_w, conv_b, lin_w, lin_b)
    nc = _get_nc()
    res = run_bass_kernel_spmd(nc, in_maps, core_ids=list(range(NCORES)))
    _LAST_RESULTS = res
    return np.concatenate([r["out"] for r in res.results], axis=0)


# revision 27
# speedup vs baseline: 3.3244x; 3.3244x over previous
"""Trainium2 Bass kernel for nn_ConvEmbedder.

out[b, p, e] = (patch(b, p) . conv_w + conv_b) * lin_w[e] + lin_b[e]

data [64, 512, 512] f32, non-overlapping 16x16 patches (1024 per image),
E = 768.  Pure data-parallel over the batch: 8 images per NeuronCore.

Per-core pipeline (per image):
  1. DMA image -> SBUF [128, 4, 512]   (row-group g holds image rows 128g..128g+127)
  2. DVE: d *= wexp  (in place)        (wexp[p, g, w] = conv_w[p%16, w%16])
  3. DVE reduce over c-groups: t3[128, 4, 32],  t3[p, g, j] = sum_c d[p, g, 16j+c]
  4. PE: 4 tiny accumulating matmuls with block-diagonal ones lhsT -> PSUM pv[32, 32]
       pv[i, j] = sum_r t3[16*(i%8)+r ...] = conv_val(patch(i, j)) - conv_b
     (fp32 matmuls cost 4 cyc/row, so keep the PE streaming dim small: the
      c-reduce runs on DVE *before* the PE row-reduce.)
  5. ScalarE copies pv -> SBUF; SBUF->SBUF DMA flattens it into lhsT[1, 0:1024]
     (patch-major); lhsT[0, :] = 1.0 via memset
  6. PE per 128-patch block: [1; v].T @ [lin_b + conv_b*lin_w; lin_w] -> PSUM [128, 768]
  7. PSUM->SBUF copy split between DVE and ScalarE
  8. contiguous 768 KB DMA store per 2 blocks
"""

import os

import numpy as np

import concourse.bacc as bacc
import concourse.tile as tile
from concourse import mybir
from concourse.bass_utils import run_bass_kernel_spmd

# fp32 matmuls stream at 4 cyc/row on the PE (2 half-speed passes); float32r
# streams at 1 cyc/row for N>=256.  The expansion matmul is a rank-2 product
# (v x lin_w + 1 x lin_b) so reduced product precision only touches ~1 ulp-ish
# terms; conv stays full fp32.  Toggle for experiments: EXP_F32R=0.
EXP_F32R = os.environ.get("EXP_F32R", "1") == "1"

KS = 16          # conv kernel == patch size
E = 768          # embed dim
NCORES = 8
B = 64
H = 512
W = 512
BPC = B // NCORES          # images per core
NH = H // KS               # 32 patch rows (and patch cols) per image
NPATCH = NH * NH           # 1024 patches per image
NG = H // 128              # 4 row-groups per image
NBLK = NPATCH // 128       # 8 expansion blocks of 128 patches
DT = mybir.dt.float32
SPLIT = 96                 # DVE copies cols [0:SPLIT), ScalarE [SPLIT:768)

_NC = None
_LAST_RESULTS = None       # BassKernelResults of the last run (for test harness)


def _build_nc(reps=None):
    # reps: bench-only — wrap the whole per-image pipeline in a HW For_i loop
    # so one NEFF launch amortizes the (huge) axon dispatch overhead.
    mm_dt = mybir.dt.float32r if EXP_F32R else DT
    nc = bacc.Bacc("TRN2", target_bir_lowering=False, debug=False)
    data_t = nc.dram_tensor("data", [BPC, H, W], DT, kind="ExternalInput")
    wexp_t = nc.dram_tensor("wexp", [128, W], DT, kind="ExternalInput")
    bd_t = nc.dram_tensor("bd", [128, NG, NH], DT, kind="ExternalInput")
    lwb_t = nc.dram_tensor("lwb", [2, E], mm_dt, kind="ExternalInput")
    ones_t = nc.dram_tensor("ones", [1, NPATCH], mm_dt, kind="ExternalInput")
    out_t = nc.dram_tensor("out", [BPC, NPATCH, E], DT, kind="ExternalOutput")

    with tile.TileContext(nc) as tc:
        with (
            tc.tile_pool(name="singles", bufs=1) as singles,
            tc.tile_pool(name="dpool", bufs=3) as dpool,
            tc.tile_pool(name="t3pool", bufs=3) as t3pool,
            tc.tile_pool(name="vpool", bufs=3) as vpool,
            tc.tile_pool(name="lpool", bufs=3) as lpool,
            tc.tile_pool(name="opool", bufs=6) as opool,
            tc.tile_pool(name="pvp", bufs=2, space="PSUM") as pvp,
            tc.tile_pool(name="pop", bufs=3, space="PSUM") as pop,
        ):
            wexp = singles.tile([128, W], DT)
            nc.sync.dma_start(out=wexp[:], in_=wexp_t.ap())
            # stride-0 broadcast of wexp across the 4 row-groups
            wexp_b = wexp[:].rearrange("p (g w) -> p g w", g=1).broadcast_to(
                [128, NG, W]
            )
            bd = singles.tile([128, NG, NH], DT)
            nc.sync.dma_start(out=bd[:], in_=bd_t.ap())
            lwb = singles.tile([2, E], mm_dt)
            nc.sync.dma_start(out=lwb[:], in_=lwb_t.ap())

            def emit_images():
              for img in range(BPC):
                # 1. load image: d[p, g, w] = data[img, 128*g + p, w]
                d = dpool.tile([128, NG, W], DT)
                nc.sync.dma_start(
                    out=d[:],
                    in_=data_t.ap()[img].rearrange("(g p) w -> p g w", p=128),
                )
                # 2. elementwise conv-weight multiply (in place)
                nc.vector.tensor_mul(d[:], d[:], wexp_b)
                # 3. reduce the 16 cols of each patch on DVE (keeps the PE
                #    streaming dim small: fp32 matmul is 4 cyc/row)
                t3 = t3pool.tile([128, NG, NH], DT)
                nc.vector.tensor_reduce(
                    out=t3[:],
                    in_=d[:].rearrange("p g (j c) -> p g j c", c=KS),
                    axis=mybir.AxisListType.X,
                    op=mybir.AluOpType.add,
                )
                # 4. reduce the 16 rows of each patch-row via block-diag ones
                pv = pvp.tile([NH, NH], DT)
                for g in range(NG):
                    nc.tensor.matmul(
                        pv[:],
                        bd[:, g, :],
                        t3[:, g, :],
                        start=(g == 0),
                        stop=(g == NG - 1),
                    )
                v8 = vpool.tile([NH, NH], mm_dt)
                nc.scalar.copy(v8[:], pv[:])
                # 5. row 0 = ones, row 1 = v flattened patch-major
                lhsT = lpool.tile([2, NPATCH], mm_dt)
                # SWDGE ring: keeps these latency-critical 4 KB moves from
                # queueing behind MiB-scale loads/stores on the HWDGE rings
                nc.gpsimd.dma_start(out=lhsT[0:1, :], in_=ones_t.ap())
                nc.gpsimd.dma_start(
                    out=lhsT[1:2, :].rearrange("o (i j) -> o i j", j=NH),
                    in_=v8[:],
                )
                # 6-8. expansion: out[p, e] = v[p]*lin_w[e] + lin_b_eff[e]
                for bb in range(0, NBLK, 2):
                    ot = opool.tile([128, 2, E], DT)
                    for sub in range(2):
                        blk = bb + sub
                        lhsT_blk = lhsT[:, 128 * blk:128 * (blk + 1)]
                        po = pop.tile([128, E], DT)
                        nc.tensor.matmul(
                            po[:, 0:512], lhsT_blk, lwb[:, 0:512],
                            start=True, stop=True,
                        )
                        nc.tensor.matmul(
                            po[:, 512:E], lhsT_blk, lwb[:, 512:E],
                            start=True, stop=True,
                        )
                        nc.vector.tensor_copy(ot[:, sub, 0:SPLIT], po[:, 0:SPLIT])
                        nc.scalar.copy(ot[:, sub, SPLIT:E], po[:, SPLIT:E])
                    nc.scalar.dma_start(
                        out=out_t.ap()[img, 128 * bb:128 * (bb + 2), :]
                        .rearrange("(blk p) e -> p blk e", p=128),
                        in_=ot[:],
                    )

            if reps is None:
                emit_images()
            else:
                with tc.For_i(0, reps, 1):
                    emit_images()
    nc.compile()
    return nc


def _get_nc():
    global _NC
    if _NC is None:
        _NC = _build_nc()
    return _NC


def _prepare_in_maps(data, conv_w, conv_b, lin_w, lin_b):
    data = np.ascontiguousarray(np.asarray(data, dtype=np.float32))
    conv_w = np.asarray(conv_w, dtype=np.float32).reshape(KS, KS)
    conv_b = np.float32(np.asarray(conv_b, dtype=np.float32))
    lin_w = np.asarray(lin_w, dtype=np.float32).reshape(E)
    lin_b = np.asarray(lin_b, dtype=np.float32).reshape(E)

    # wexp[p, w] = conv_w[p % 16, w % 16]
    wexp = np.ascontiguousarray(np.tile(conv_w, (128 // KS, W // KS)))
    # bd[row, g, m] = 1 iff m == 8*g + row//16  (block-diagonal ones)
    bd = np.zeros((128, NG, NH), dtype=np.float32)
    rows = np.arange(128)
    for g in range(NG):
        bd[rows, g, (128 // KS) * g + rows // KS] = 1.0
    # fold conv_b: v*lin_w + (conv_b*lin_w + lin_b)
    lin_b_eff = (
        np.float64(conv_b) * lin_w.astype(np.float64) + lin_b.astype(np.float64)
    ).astype(np.float32)
    lwb = np.ascontiguousarray(np.stack([lin_b_eff, lin_w], axis=0))
    ones = np.ones((1, NPATCH), dtype=np.float32)

    return [
        {
            "data": np.ascontiguousarray(data[i * BPC:(i + 1) * BPC]),
            "wexp": wexp,
            "bd": bd,
            "lwb": lwb,
            "ones": ones,
        }
        for i in range(NCORES)
    ]


def kernel(data, conv_w, conv_b, lin_w, lin_b):
    global _LAST_RESULTS
    in_maps = _prepare_in_maps(data, conv_w, conv_b, lin_w, lin_b)
    nc = _get_nc()
    res = run_bass_kernel_spmd(nc, in_maps, core_ids=list(range(NCORES)))
    _LAST_RESULTS = res
    return np.concatenate([r["out"] for r in res.results], axis=0)
